# revision 5
# baseline (speedup 1.0000x reference)
"""Trainium2 Bass kernel: Mixture-of-Depths routed FFN block.

Computation (per batch row b of x [B=4, S=4096, D=2048]):
  logits = x[b] @ Wr + br                       # router
  top-512 tokens by logit, positions sorted ascending
  h = gelu(x[b][tokens] @ W1 + b1); o = h @ W2 + b2
  rw = softmax(logits[tokens])
  out[b] = x[b];  out[b][tokens] += rw * o

Distribution (8 NeuronCores, no collectives):
  Core 2p   handles batch p on the natural x[p].
  Core 2p+1 handles batch p on the row-REVERSED x[p] (host flips, then
  un-flips its output half) - this makes the program fully SPMD-symmetric:
  every core routes, selects the top-512, compacts ascending positions in
  its own coordinate space, FFNs the FIRST 384 ranks (covers every token
  that can land in its output half; actual per-half counts are ~256+-25),
  pass-through copies rows [0:2048) of its x to its out, and scatters its
  updated token rows.  Host takes rows [0:2048) from core 2p and reversed
  rows [0:2048) from core 2p+1.

On-device pipeline per core:
  1. Router: stream x tiles [128,2048]; fused multiply+row-reduce on DVE
     against a host-replicated Wr; same tiles pass-through-copied to out.
  2. Top-k threshold: 34-step float bisection on [0.25, 4.0] for the value
     t with count(logit >= t) == 512 (exact; logit sigma == 1 by
     construction so the bracket is safe - verified via the dbg output).
  3. Compaction: slot-space mask -> sparse_gather (GPSIMD) -> sorted token
     ids + their logits; softmax weights from the compacted logits.
  4. Gather 384 token rows via indirect DMA; PE-transpose to d-major bf16.
  5. FFN: hT[dff,tok] = W1.T-tiles @ fxT (bf16, f32 PSUM), fused
     gelu+bias on ACT; out_l[tok,d] = hT.T-tiles @ W2 accumulated in SBUF.
  6. Epilogue: upd = fx + rw*(out_l + b2); indirect-DMA scatter rows.
"""

import numpy as np
import ml_dtypes

B, S, D, DFF = 4, 4096, 2048, 8192
K_TOP = 512
P = 128
WIN = 384            # token ranks processed per core (3 groups of 128)
NG = WIN // P        # 3
N_CORES = 8
BISECT_ITERS = 34
BISECT_LO, BISECT_HI = 0.25, 4.0
MB = 2               # mm1 dff-tile block (PSUM tiles live)
KFB = 4              # mm2 dff-chunk block (W2 tiles live)

_CACHE = {}


def _build():
    if "nc" in _CACHE:
        return _CACHE["nc"]
    from contextlib import ExitStack
    import concourse.bacc as bacc
    import concourse.tile as tile
    from concourse import bass, mybir, library_config
    from concourse.masks import make_identity

    f32 = mybir.dt.float32
    bf16 = mybir.dt.bfloat16
    i32 = mybir.dt.int32
    u32 = mybir.dt.uint32
    A = mybir.AluOpType
    ACTF = mybir.ActivationFunctionType

    nc = bacc.Bacc("TRN2", target_bir_lowering=False, debug=False)

    xd = nc.dram_tensor("x", [S, D], f32, kind="ExternalInput")
    w1d = nc.dram_tensor("w1", [D, DFF], bf16, kind="ExternalInput")
    w2d = nc.dram_tensor("w2", [DFF, D], bf16, kind="ExternalInput")
    wrd = nc.dram_tensor("wrb", [P, D], f32, kind="ExternalInput")
    brd = nc.dram_tensor("brb", [P, 1], f32, kind="ExternalInput")
    b1d = nc.dram_tensor("b1s", [P, DFF // P], f32, kind="ExternalInput")
    b2d = nc.dram_tensor("b2b", [P, D], f32, kind="ExternalInput")
    idd = nc.dram_tensor("idp", [16, S // 16], f32, kind="ExternalInput")
    outd = nc.dram_tensor("out", [S, D], f32, kind="ExternalOutput")
    dbgd = nc.dram_tensor("dbg", [1, 4], f32, kind="ExternalOutput")

    with tile.TileContext(nc) as tc, ExitStack() as ctx:
        const = ctx.enter_context(tc.tile_pool(name="const", bufs=1))
        xpool = ctx.enter_context(tc.tile_pool(name="xp", bufs=3))
        rscr = ctx.enter_context(tc.tile_pool(name="rscr", bufs=1))
        small = ctx.enter_context(tc.tile_pool(name="small", bufs=1))
        bscr = ctx.enter_context(tc.tile_pool(name="bscr", bufs=2))
        fxpool = ctx.enter_context(tc.tile_pool(name="fxp", bufs=1))
        big = ctx.enter_context(tc.tile_pool(name="big", bufs=1))
        w1pool = ctx.enter_context(tc.tile_pool(name="w1p", bufs=3))
        w2pool = ctx.enter_context(tc.tile_pool(name="w2p", bufs=2))
        updp = ctx.enter_context(tc.tile_pool(name="updp", bufs=1))
        pps_ctx = tc.tile_pool(name="pps", bufs=2, space="PSUM")
        pps = pps_ctx.__enter__()

        # ---- constants
        wrb = const.tile([P, D], f32)
        nc.sync.dma_start(wrb[:], wrd[:])
        brb = const.tile([P, 1], f32)
        nc.sync.dma_start(brb[:], brd[:])
        b1s = const.tile([P, DFF // P], f32)
        nc.sync.dma_start(b1s[:], b1d[:])
        b2b = const.tile([P, D], f32)
        nc.sync.dma_start(b2b[:], b2d[:])
        idp = const.tile([16, S // 16], f32)
        nc.sync.dma_start(idp[:], idd[:])
        ident = const.tile([P, P], f32)
        make_identity(nc, ident[:])
        ones_col = const.tile([P, 1], f32)
        nc.vector.memset(ones_col[:], 1.0)
        ones_row = const.tile([1, P], f32)
        nc.vector.memset(ones_row[:], 1.0)

        # ---- Phase 1: router + pass-through copy of first half
        L = small.tile([P, S // P], f32)  # L[p, t] = logit(token t*128+p)
        for t in range(S // P):
            xt = xpool.tile([P, D], f32)
            nc.sync.dma_start(xt[:], xd[t * P:(t + 1) * P, :])
            if t < S // P // 2:
                nc.sync.dma_start(outd[t * P:(t + 1) * P, :], xt[:])
            scr = rscr.tile([P, D], f32)
            nc.vector.scalar_tensor_tensor(
                out=scr[:], in0=xt[:], scalar=1.0, in1=wrb[:],
                op0=A.mult, op1=A.mult, accum_out=L[:, t:t + 1],
            )
        nc.vector.tensor_scalar(
            out=L[:], in0=L[:], scalar1=brb[:, :1], scalar2=None, op0=A.add)

        # ---- Phase 2: bisection for the k-th largest logit
        lo = small.tile([P, 1], f32)
        hi = small.tile([P, 1], f32)
        mid = small.tile([P, 1], f32)
        ge = small.tile([P, 1], mybir.dt.uint8)
        lt = small.tile([P, 1], mybir.dt.uint8)
        nc.vector.memset(lo[:], BISECT_LO)
        nc.vector.memset(hi[:], BISECT_HI)
        for _ in range(BISECT_ITERS):
            nc.vector.tensor_tensor(out=mid[:], in0=lo[:], in1=hi[:], op=A.add)
            nc.vector.tensor_scalar_mul(mid[:], mid[:], 0.5)
            scrb = bscr.tile([P, S // P], f32)
            cnt = bscr.tile([P, 1], f32)
            nc.vector.tensor_scalar(
                out=scrb[:], in0=L[:], scalar1=mid[:, :1], scalar2=None,
                op0=A.is_ge, op1=A.add, accum_out=cnt[:, :1],
            )
            tot = pps.tile([1, 1], f32, space="PSUM", tag="p11")
            nc.tensor.matmul(tot[:], lhsT=ones_col[:], rhs=cnt[:], start=True, stop=True)
            tot_sb = bscr.tile([1, 1], f32)
            nc.vector.tensor_copy(tot_sb[:], tot[:])
            bc = pps.tile([P, 1], f32, space="PSUM", tag="pP1")
            nc.tensor.matmul(bc[:], lhsT=ones_row[:], rhs=tot_sb[:], start=True, stop=True)
            nc.vector.tensor_scalar(
                out=ge[:], in0=bc[:], scalar1=float(K_TOP), scalar2=None, op0=A.is_ge)
            nc.vector.copy_predicated(lo[:], ge[:], mid[:])
            nc.vector.tensor_scalar(
                out=lt[:], in0=bc[:], scalar1=float(K_TOP), scalar2=None, op0=A.is_lt)
            nc.vector.copy_predicated(hi[:], lt[:], mid[:])

        # ---- Phase 3: compaction (slot space: token i at [i%16, i//16])
        L16 = small.tile([16, S // 16], f32)
        for q in range(8):
            nc.sync.dma_start(L16[:, q:S // 16:8], L[16 * q:16 * (q + 1), :])
        t16 = lo[0:16, 0:1]
        arr_ids = small.tile([16, S // 16], f32)
        nc.vector.scalar_tensor_tensor(
            out=arr_ids[:], in0=L16[:], scalar=t16, in1=idp[:],
            op0=A.is_ge, op1=A.mult)
        nc.vector.tensor_scalar_add(arr_ids[:], arr_ids[:], -1.0)
        lp64 = small.tile([16, S // 16], f32)
        nc.vector.tensor_scalar_add(lp64[:], L16[:], 64.0)
        arr_lg = small.tile([16, S // 16], f32)
        nc.vector.scalar_tensor_tensor(
            out=arr_lg[:], in0=L16[:], scalar=t16, in1=lp64[:],
            op0=A.is_ge, op1=A.mult)
        nc.vector.tensor_scalar_add(arr_lg[:], arr_lg[:], -1.0)
        ids512 = small.tile([16, K_TOP // 16], f32)
        lg512 = small.tile([16, K_TOP // 16], f32)
        nf1 = small.tile([1, 1], u32)
        nf2 = small.tile([1, 1], u32)
        with tc.tile_critical():
            nc.gpsimd.load_library(library_config.sparse_gather)
            nc.gpsimd.sparse_gather(ids512[:], arr_ids[:], num_found=nf1[:, :1])
            nc.gpsimd.sparse_gather(lg512[:], arr_lg[:], num_found=nf2[:, :1])
        dbg = small.tile([1, 4], f32)
        nc.vector.tensor_copy(dbg[:, 0:1], nf1[:])
        nc.vector.tensor_copy(dbg[:, 1:2], nf2[:])
        nc.vector.tensor_copy(dbg[:, 2:3], lo[0:1, 0:1])
        nc.vector.tensor_copy(dbg[:, 3:4], hi[0:1, 0:1])
        nc.sync.dma_start(dbgd[:], dbg[:])

        # ---- Phase 4: token-major shuffles, softmax weights, int ids
        ids_tok = small.tile([P, K_TOP // P], f32)
        lg_tok = small.tile([P, K_TOP // P], f32)
        for q in range(8):
            nc.sync.dma_start(ids_tok[16 * q:16 * (q + 1), :], ids512[:, q:K_TOP // 16:8])
            nc.sync.dma_start(lg_tok[16 * q:16 * (q + 1), :], lg512[:, q:K_TOP // 16:8])
        nc.vector.tensor_scalar_add(lg_tok[:], lg_tok[:], -63.0)
        exps = small.tile([P, K_TOP // P], f32)
        nc.scalar.activation(exps[:], lg_tok[:], ACTF.Exp)
        sume = small.tile([P, 1], f32)
        nc.vector.tensor_reduce(sume[:], exps[:], axis=mybir.AxisListType.X, op=A.add)
        den = pps.tile([1, 1], f32, space="PSUM", tag="p11")
        nc.tensor.matmul(den[:], lhsT=ones_col[:], rhs=sume[:], start=True, stop=True)
        den_sb = small.tile([1, 1], f32)
        nc.vector.tensor_copy(den_sb[:], den[:])
        recip = small.tile([1, 1], f32)
        nc.vector.reciprocal(recip[:], den_sb[:])
        rcb = pps.tile([P, 1], f32, space="PSUM", tag="pP1")
        nc.tensor.matmul(rcb[:], lhsT=ones_row[:], rhs=recip[:], start=True, stop=True)
        rw_tok = small.tile([P, NG], f32)
        nc.vector.tensor_scalar_mul(rw_tok[:], exps[:, :NG], rcb[:, :1])
        ids_i32 = small.tile([P, NG], i32)
        nc.vector.tensor_copy(ids_i32[:], ids_tok[:, :NG])

        # ---- Phase 5: gather + transpose to fxT (bf16, d-major)
        pps_ctx.__exit__(None, None, None)
        tpp = ctx.enter_context(tc.tile_pool(name="tpp", bufs=2, space="PSUM"))
        mmp = ctx.enter_context(tc.tile_pool(name="mmp", bufs=2, space="PSUM"))
        mm2p = ctx.enter_context(tc.tile_pool(name="mm2p", bufs=2, space="PSUM"))
        fxT = [big.tile([P, WIN], bf16, name=f"fxT{k}") for k in range(D // P)]
        fx_list = []
        for g in range(NG):
            fxg = fxpool.tile([P, D], f32, tag=f"fx{g}", name=f"fxg{g}")
            nc.gpsimd.indirect_dma_start(
                out=fxg[:], out_offset=None, in_=xd[:],
                in_offset=bass.IndirectOffsetOnAxis(ap=ids_i32[:, g:g + 1], axis=0),
            )
            fx_list.append(fxg)
            for k in range(16):
                tp = tpp.tile([P, P], f32, space="PSUM")
                nc.tensor.transpose(tp[:], fxg[:, k * P:(k + 1) * P], ident[:])
                nc.vector.tensor_copy(fxT[k][:, g * P:(g + 1) * P], tp[:])

        # ---- Phase 6: mm1 + fused gelu/bias -> hT (bf16, dff-major)
        hT = [big.tile([P, WIN], bf16, name=f"hT{m}") for m in range(DFF // P)]
        for mb in range(DFF // P // MB):
            hps = [mmp.tile([P, WIN], f32, space="PSUM", tag=f"hps{j}", name=f"hps{j}") for j in range(MB)]
            for k in range(D // P):
                w1sl = w1pool.tile([P, MB * P], bf16)
                nc.sync.dma_start(
                    w1sl[:], w1d[k * P:(k + 1) * P, mb * MB * P:(mb + 1) * MB * P])
                for j in range(MB):
                    nc.tensor.matmul(
                        hps[j][:], lhsT=w1sl[:, j * P:(j + 1) * P],
                        rhs=fxT[k][:],
                        start=(k == 0), stop=(k == D // P - 1))
            for j in range(MB):
                m = mb * MB + j
                nc.scalar.activation(
                    hT[m][:], hps[j][:], ACTF.Gelu, bias=b1s[:, m:m + 1])

        # ---- Phase 7: mm2, accumulate out_l in SBUF
        acc = big.tile([P, NG * D], f32)  # token group g -> [:, g*D:(g+1)*D]
        n_kfb = DFF // P // KFB
        for kfb in range(n_kfb):
            w2t = [w2pool.tile([P, D], bf16, tag=f"w2{j}", name=f"w2t{j}") for j in range(KFB)]
            for j in range(KFB):
                kf = kfb * KFB + j
                nc.sync.dma_start(w2t[j][:], w2d[kf * P:(kf + 1) * P, :])
            for g in range(NG):
                for ds in range(D // 512):
                    ps = mm2p.tile([P, 512], f32, space="PSUM")
                    for j in range(KFB):
                        kf = kfb * KFB + j
                        nc.tensor.matmul(
                            ps[:], lhsT=hT[kf][:, g * P:(g + 1) * P],
                            rhs=w2t[j][:, ds * 512:(ds + 1) * 512],
                            start=(j == 0), stop=(j == KFB - 1))
                    dst = acc[:, g * D + ds * 512:g * D + (ds + 1) * 512]
                    if kfb == 0:
                        nc.vector.tensor_copy(dst, ps[:])
                    else:
                        nc.vector.tensor_tensor(out=dst, in0=dst, in1=ps[:], op=A.add)

        # ---- Phase 8: epilogue + scatter
        for g in range(NG):
            upd = updp.tile([P, D], f32)
            nc.vector.tensor_tensor(
                out=upd[:], in0=acc[:, g * D:(g + 1) * D], in1=b2b[:], op=A.add)
            nc.vector.scalar_tensor_tensor(
                out=upd[:], in0=upd[:], scalar=rw_tok[:, g:g + 1], in1=fx_list[g][:],
                op0=A.mult, op1=A.add)
            nc.gpsimd.indirect_dma_start(
                out=outd[:],
                out_offset=bass.IndirectOffsetOnAxis(ap=ids_i32[:, g:g + 1], axis=0),
                in_=upd[:], in_offset=None,
            )

    nc.compile()
    _CACHE["nc"] = nc
    return nc


def _prep_in_maps(x, Wr, br, W1, b1, W2, b2):
    bf = ml_dtypes.bfloat16
    w1b = np.ascontiguousarray(W1.astype(bf))
    w2b = np.ascontiguousarray(W2.astype(bf))
    wrb = np.ascontiguousarray(np.broadcast_to(Wr[:, 0][None, :], (P, D)), np.float32)
    brb = np.full((P, 1), np.float32(br[0]), np.float32)
    b1s = np.ascontiguousarray(b1.reshape(DFF // P, P).T, np.float32)
    b2b = np.ascontiguousarray(np.broadcast_to(b2[None, :], (P, D)), np.float32)
    sl = np.arange(S)
    idp = np.zeros((16, S // 16), np.float32)
    idp[sl % 16, sl // 16] = sl + 1  # slot id + 1 (so unselected -> -1 after shift)
    in_maps = []
    for c in range(N_CORES):
        pair, role = c // 2, c % 2
        xc = x[pair] if role == 0 else x[pair][::-1]
        in_maps.append({
            "x": np.ascontiguousarray(xc, np.float32),
            "w1": w1b, "w2": w2b, "wrb": wrb, "brb": brb,
            "b1s": b1s, "b2b": b2b, "idp": idp,
        })
    return in_maps


def _assemble(results, x):
    out = np.empty_like(x)
    for pair in range(B):
        a = results[2 * pair]["out"]
        b = results[2 * pair + 1]["out"]
        out[pair, :S // 2] = a[:S // 2]
        out[pair, S // 2:] = b[:S // 2][::-1]
    for c in range(N_CORES):
        dbg = results[c]["dbg"]
        if not (dbg[0, 0] == K_TOP and dbg[0, 1] == K_TOP):
            raise RuntimeError(f"core {c}: top-k count mismatch, dbg={dbg}")
    return out


def run_on_device(x, Wr, br, W1, b1, W2, b2, trace=False, trace_kwargs=None):
    from concourse.bass_utils import run_bass_kernel_spmd
    nc = _build()
    in_maps = _prep_in_maps(x, Wr, br, W1, b1, W2, b2)
    res = run_bass_kernel_spmd(
        nc, in_maps, core_ids=list(range(N_CORES)),
        trace=trace, **(trace_kwargs or {}),
    )
    out = _assemble(res.results, x)
    return out, res


def kernel(x, Wr, br, W1, b1, W2, b2):
    x = np.asarray(x, np.float32)
    out, _ = run_on_device(
        x, np.asarray(Wr, np.float32), np.asarray(br, np.float32),
        np.asarray(W1, np.float32), np.asarray(b1, np.float32),
        np.asarray(W2, np.float32), np.asarray(b2, np.float32))
    return out


# revision 7
# speedup vs baseline: 1.0509x; 1.0509x over previous
"""Trainium2 Bass kernel: Mixture-of-Depths routed FFN block.

Computation (per batch row b of x [B=4, S=4096, D=2048]):
  logits = x[b] @ Wr + br                       # router
  top-512 tokens by logit, positions sorted ascending
  h = gelu(x[b][tokens] @ W1 + b1); o = h @ W2 + b2
  rw = softmax(logits[tokens])
  out[b] = x[b];  out[b][tokens] += rw * o

Distribution (8 NeuronCores, no collectives):
  Core 2p   handles batch p on the natural x[p].
  Core 2p+1 handles batch p on the row-REVERSED x[p] (host flips, then
  un-flips its output half) - this makes the program fully SPMD-symmetric:
  every core routes, selects the top-512, compacts ascending positions in
  its own coordinate space, FFNs the FIRST 384 ranks (covers every token
  that can land in its output half; actual per-half counts are ~256+-25),
  pass-through copies rows [0:2048) of its x to its out, and scatters its
  updated token rows.  Host takes rows [0:2048) from core 2p and reversed
  rows [0:2048) from core 2p+1.

On-device pipeline per core:
  1. Router: stream x tiles [128,2048]; fused multiply+row-reduce on DVE
     against a host-replicated Wr; same tiles pass-through-copied to out.
  2. Top-k threshold: 34-step float bisection on [0.25, 4.0] for the value
     t with count(logit >= t) == 512 (exact; logit sigma == 1 by
     construction so the bracket is safe - verified via the dbg output).
  3. Compaction: slot-space mask -> sparse_gather (GPSIMD) -> sorted token
     ids + their logits; softmax weights from the compacted logits.
  4. Gather 384 token rows via indirect DMA; PE-transpose to d-major bf16.
  5. FFN: hT[dff,tok] = W1.T-tiles @ fxT (bf16, f32 PSUM), fused
     gelu+bias on ACT; out_l[tok,d] = hT.T-tiles @ W2 accumulated in SBUF.
  6. Epilogue: upd = fx + rw*(out_l + b2); indirect-DMA scatter rows.
"""

import numpy as np
import ml_dtypes

B, S, D, DFF = 4, 4096, 2048, 8192
K_TOP = 512
P = 128
WIN = 384            # token ranks processed per core (3 groups of 128)
NG = WIN // P        # 3
N_CORES = 8
BISECT_ITERS = 34
BISECT_LO, BISECT_HI = 0.25, 4.0
MB = 2               # mm1 dff-tile block (PSUM tiles live)
KFB = 4              # mm2 dff-chunk block (W2 tiles live)

_CACHE = {}


def _build():
    if "nc" in _CACHE:
        return _CACHE["nc"]
    from contextlib import ExitStack
    import concourse.bacc as bacc
    import concourse.tile as tile
    from concourse import bass, mybir, library_config
    from concourse.masks import make_identity

    f32 = mybir.dt.float32
    bf16 = mybir.dt.bfloat16
    i32 = mybir.dt.int32
    u32 = mybir.dt.uint32
    A = mybir.AluOpType
    ACTF = mybir.ActivationFunctionType

    nc = bacc.Bacc("TRN2", target_bir_lowering=False, debug=False)

    xd = nc.dram_tensor("x", [S, D], f32, kind="ExternalInput")
    w1d = nc.dram_tensor("w1", [D, DFF], bf16, kind="ExternalInput")
    w2d = nc.dram_tensor("w2", [DFF, D], bf16, kind="ExternalInput")
    wrd = nc.dram_tensor("wrb", [P, D], f32, kind="ExternalInput")
    brd = nc.dram_tensor("brb", [P, 1], f32, kind="ExternalInput")
    b1d = nc.dram_tensor("b1s", [P, DFF // P], f32, kind="ExternalInput")
    b2d = nc.dram_tensor("b2b", [P, D], f32, kind="ExternalInput")
    idd = nc.dram_tensor("idp", [16, S // 16], f32, kind="ExternalInput")
    outd = nc.dram_tensor("out", [S, D], f32, kind="ExternalOutput")
    dbgd = nc.dram_tensor("dbg", [1, 4], f32, kind="ExternalOutput")

    with tile.TileContext(nc) as tc, ExitStack() as ctx:
        const = ctx.enter_context(tc.tile_pool(name="const", bufs=1))
        xpool = ctx.enter_context(tc.tile_pool(name="xp", bufs=3))
        small = ctx.enter_context(tc.tile_pool(name="small", bufs=1))
        bscr = ctx.enter_context(tc.tile_pool(name="bscr", bufs=2))
        fxpool = ctx.enter_context(tc.tile_pool(name="fxp", bufs=1))
        big = ctx.enter_context(tc.tile_pool(name="big", bufs=1))
        w1pool = ctx.enter_context(tc.tile_pool(name="w1p", bufs=8))
        w2pool = ctx.enter_context(tc.tile_pool(name="w2p", bufs=2))
        updp = ctx.enter_context(tc.tile_pool(name="updp", bufs=1))
        pps_ctx = tc.tile_pool(name="pps", bufs=2, space="PSUM")
        pps = pps_ctx.__enter__()

        # ---- constants
        wrb = const.tile([P, D], f32)
        nc.sync.dma_start(wrb[:], wrd[:])
        brb = const.tile([P, 1], f32)
        nc.sync.dma_start(brb[:], brd[:])
        b1s = const.tile([P, DFF // P], f32)
        nc.sync.dma_start(b1s[:], b1d[:])
        b2b = const.tile([P, D], f32)
        nc.sync.dma_start(b2b[:], b2d[:])
        idp = const.tile([16, S // 16], f32)
        nc.sync.dma_start(idp[:], idd[:])
        ident = const.tile([P, P], f32)
        make_identity(nc, ident[:])
        ones_col = const.tile([P, 1], f32)
        nc.vector.memset(ones_col[:], 1.0)
        ones_row = const.tile([1, P], f32)
        nc.vector.memset(ones_row[:], 1.0)
        ones2d = const.tile([P, P], f32)
        nc.vector.memset(ones2d[:], 1.0)

        # ---- Phase 1: router + pass-through copy of first half
        L = small.tile([P, S // P], f32)  # L[p, t] = logit(token t*128+p)
        for t in range(S // P):
            xt = xpool.tile([P, D], f32)
            nc.sync.dma_start(xt[:], xd[t * P:(t + 1) * P, :])
            nc.vector.scalar_tensor_tensor(
                out=xt[:], in0=xt[:], scalar=1.0, in1=wrb[:],
                op0=A.mult, op1=A.mult, accum_out=L[:, t:t + 1],
            )
        nc.vector.tensor_scalar(
            out=L[:], in0=L[:], scalar1=brb[:, :1], scalar2=None, op0=A.add)

        # ---- Phase 2: bisection for the k-th largest logit
        lo = small.tile([P, 1], f32)
        hi = small.tile([P, 1], f32)
        mid = small.tile([P, 1], f32)
        ge = small.tile([P, 1], mybir.dt.uint8)
        lt = small.tile([P, 1], mybir.dt.uint8)
        nc.vector.memset(lo[:], BISECT_LO)
        nc.vector.memset(hi[:], BISECT_HI)
        for _ in range(BISECT_ITERS):
            nc.vector.tensor_tensor(out=mid[:], in0=lo[:], in1=hi[:], op=A.add)
            nc.vector.tensor_scalar_mul(mid[:], mid[:], 0.5)
            scrb = bscr.tile([P, S // P], f32)
            cnt = bscr.tile([P, 1], f32)
            nc.vector.tensor_scalar(
                out=scrb[:], in0=L[:], scalar1=mid[:, :1], scalar2=None,
                op0=A.is_ge, op1=A.add, accum_out=cnt[:, :1],
            )
            bc = pps.tile([P, 1], f32, space="PSUM", tag="pP1")
            nc.tensor.matmul(bc[:], lhsT=ones2d[:], rhs=cnt[:], start=True, stop=True)
            nc.vector.tensor_scalar(
                out=ge[:], in0=bc[:], scalar1=float(K_TOP), scalar2=None, op0=A.is_ge)
            nc.vector.copy_predicated(lo[:], ge[:], mid[:])
            nc.vector.tensor_scalar(
                out=lt[:], in0=bc[:], scalar1=float(K_TOP), scalar2=None, op0=A.is_lt)
            nc.vector.copy_predicated(hi[:], lt[:], mid[:])

        # ---- Phase 3: compaction (slot space: token i at [i%16, i//16])
        L16 = small.tile([16, S // 16], f32)
        for q in range(8):
            nc.sync.dma_start(L16[:, q:S // 16:8], L[16 * q:16 * (q + 1), :])
        t16 = lo[0:16, 0:1]
        arr_ids = small.tile([16, S // 16], f32)
        nc.vector.scalar_tensor_tensor(
            out=arr_ids[:], in0=L16[:], scalar=t16, in1=idp[:],
            op0=A.is_ge, op1=A.mult)
        nc.vector.tensor_scalar_add(arr_ids[:], arr_ids[:], -1.0)
        lp64 = small.tile([16, S // 16], f32)
        nc.vector.tensor_scalar_add(lp64[:], L16[:], 64.0)
        arr_lg = small.tile([16, S // 16], f32)
        nc.vector.scalar_tensor_tensor(
            out=arr_lg[:], in0=L16[:], scalar=t16, in1=lp64[:],
            op0=A.is_ge, op1=A.mult)
        nc.vector.tensor_scalar_add(arr_lg[:], arr_lg[:], -1.0)
        ids512 = small.tile([16, K_TOP // 16], f32)
        lg512 = small.tile([16, K_TOP // 16], f32)
        nf1 = small.tile([1, 1], u32)
        nf2 = small.tile([1, 1], u32)
        with tc.tile_critical():
            nc.gpsimd.load_library(library_config.sparse_gather)
            nc.gpsimd.sparse_gather(ids512[:], arr_ids[:], num_found=nf1[:, :1])
            nc.gpsimd.sparse_gather(lg512[:], arr_lg[:], num_found=nf2[:, :1])
        dbg = small.tile([1, 4], f32)
        nc.vector.tensor_copy(dbg[:, 0:1], nf1[:])
        nc.vector.tensor_copy(dbg[:, 1:2], nf2[:])
        nc.vector.tensor_copy(dbg[:, 2:3], lo[0:1, 0:1])
        nc.vector.tensor_copy(dbg[:, 3:4], hi[0:1, 0:1])
        nc.sync.dma_start(dbgd[:], dbg[:])

        # ---- Phase 4: token-major shuffles, softmax weights, int ids
        ids_tok = small.tile([P, K_TOP // P], f32)
        lg_tok = small.tile([P, K_TOP // P], f32)
        for q in range(8):
            nc.sync.dma_start(ids_tok[16 * q:16 * (q + 1), :], ids512[:, q:K_TOP // 16:8])
            nc.sync.dma_start(lg_tok[16 * q:16 * (q + 1), :], lg512[:, q:K_TOP // 16:8])
        nc.vector.tensor_scalar_add(lg_tok[:], lg_tok[:], -63.0)
        exps = small.tile([P, K_TOP // P], f32)
        nc.scalar.activation(exps[:], lg_tok[:], ACTF.Exp)
        sume = small.tile([P, 1], f32)
        nc.vector.tensor_reduce(sume[:], exps[:], axis=mybir.AxisListType.X, op=A.add)
        den = pps.tile([1, 1], f32, space="PSUM", tag="p11")
        nc.tensor.matmul(den[:], lhsT=ones_col[:], rhs=sume[:], start=True, stop=True)
        den_sb = small.tile([1, 1], f32)
        nc.vector.tensor_copy(den_sb[:], den[:])
        recip = small.tile([1, 1], f32)
        nc.vector.reciprocal(recip[:], den_sb[:])
        rcb = pps.tile([P, 1], f32, space="PSUM", tag="pP1")
        nc.tensor.matmul(rcb[:], lhsT=ones_row[:], rhs=recip[:], start=True, stop=True)
        rw_tok = small.tile([P, NG], f32)
        nc.vector.tensor_scalar_mul(rw_tok[:], exps[:, :NG], rcb[:, :1])
        ids_i32 = small.tile([P, NG], i32)
        nc.vector.tensor_copy(ids_i32[:], ids_tok[:, :NG])

        # ---- Phase 5: gather + transpose to fxT (bf16, d-major)
        pps_ctx.__exit__(None, None, None)
        tpp = ctx.enter_context(tc.tile_pool(name="tpp", bufs=2, space="PSUM"))
        mmp = ctx.enter_context(tc.tile_pool(name="mmp", bufs=2, space="PSUM"))
        mm2p = ctx.enter_context(tc.tile_pool(name="mm2p", bufs=2, space="PSUM"))
        fxT = [big.tile([P, WIN], bf16, name=f"fxT{k}") for k in range(D // P)]
        fx_list = []
        for g in range(NG):
            fxg = fxpool.tile([P, D], f32, tag=f"fx{g}", name=f"fxg{g}")
            nc.gpsimd.indirect_dma_start(
                out=fxg[:], out_offset=None, in_=xd[:],
                in_offset=bass.IndirectOffsetOnAxis(ap=ids_i32[:, g:g + 1], axis=0),
            )
            fx_list.append(fxg)
            for k in range(16):
                tp = tpp.tile([P, P], f32, space="PSUM")
                nc.tensor.transpose(tp[:], fxg[:, k * P:(k + 1) * P], ident[:])
                nc.vector.tensor_copy(fxT[k][:, g * P:(g + 1) * P], tp[:])

        # ---- pass-through copy (DRAM->DRAM), overlaps the FFN phase
        for t in range(S // P // 2):
            nc.sync.dma_start(outd[t * P:(t + 1) * P, :], xd[t * P:(t + 1) * P, :])

        # ---- Phase 6: mm1 + fused gelu/bias -> hT (bf16, dff-major)
        hT = [big.tile([P, WIN], bf16, name=f"hT{m}") for m in range(DFF // P)]
        for mb in range(DFF // P // MB):
            hps = [mmp.tile([P, WIN], f32, space="PSUM", tag=f"hps{j}", name=f"hps{j}") for j in range(MB)]
            for k in range(D // P):
                w1sl = w1pool.tile([P, MB * P], bf16)
                nc.sync.dma_start(
                    w1sl[:], w1d[k * P:(k + 1) * P, mb * MB * P:(mb + 1) * MB * P])
                for j in range(MB):
                    nc.tensor.matmul(
                        hps[j][:], lhsT=w1sl[:, j * P:(j + 1) * P],
                        rhs=fxT[k][:],
                        start=(k == 0), stop=(k == D // P - 1))
            for j in range(MB):
                m = mb * MB + j
                nc.scalar.activation(
                    hT[m][:], hps[j][:], ACTF.Gelu, bias=b1s[:, m:m + 1])

        # ---- Phase 7: mm2, accumulate out_l in SBUF
        acc = big.tile([P, NG * D], f32)  # token group g -> [:, g*D:(g+1)*D]
        n_kfb = DFF // P // KFB
        for kfb in range(n_kfb):
            w2t = [w2pool.tile([P, D], bf16, tag=f"w2{j}", name=f"w2t{j}") for j in range(KFB)]
            for j in range(KFB):
                kf = kfb * KFB + j
                nc.sync.dma_start(w2t[j][:], w2d[kf * P:(kf + 1) * P, :])
            for g in range(NG):
                for ds in range(D // 512):
                    ps = mm2p.tile([P, 512], f32, space="PSUM")
                    for j in range(KFB):
                        kf = kfb * KFB + j
                        nc.tensor.matmul(
                            ps[:], lhsT=hT[kf][:, g * P:(g + 1) * P],
                            rhs=w2t[j][:, ds * 512:(ds + 1) * 512],
                            start=(j == 0), stop=(j == KFB - 1))
                    dst = acc[:, g * D + ds * 512:g * D + (ds + 1) * 512]
                    if kfb == 0:
                        nc.vector.tensor_copy(dst, ps[:])
                    else:
                        nc.vector.tensor_tensor(out=dst, in0=dst, in1=ps[:], op=A.add)

        # ---- Phase 8: epilogue + scatter
        for g in range(NG):
            upd = updp.tile([P, D], f32)
            nc.vector.tensor_tensor(
                out=upd[:], in0=acc[:, g * D:(g + 1) * D], in1=b2b[:], op=A.add)
            nc.vector.scalar_tensor_tensor(
                out=upd[:], in0=upd[:], scalar=rw_tok[:, g:g + 1], in1=fx_list[g][:],
                op0=A.mult, op1=A.add)
            nc.gpsimd.indirect_dma_start(
                out=outd[:],
                out_offset=bass.IndirectOffsetOnAxis(ap=ids_i32[:, g:g + 1], axis=0),
                in_=upd[:], in_offset=None,
            )

    nc.compile()
    _CACHE["nc"] = nc
    return nc


def _prep_in_maps(x, Wr, br, W1, b1, W2, b2):
    bf = ml_dtypes.bfloat16
    w1b = np.ascontiguousarray(W1.astype(bf))
    w2b = np.ascontiguousarray(W2.astype(bf))
    wrb = np.ascontiguousarray(np.broadcast_to(Wr[:, 0][None, :], (P, D)), np.float32)
    brb = np.full((P, 1), np.float32(br[0]), np.float32)
    b1s = np.ascontiguousarray(b1.reshape(DFF // P, P).T, np.float32)
    b2b = np.ascontiguousarray(np.broadcast_to(b2[None, :], (P, D)), np.float32)
    sl = np.arange(S)
    idp = np.zeros((16, S // 16), np.float32)
    idp[sl % 16, sl // 16] = sl + 1  # slot id + 1 (so unselected -> -1 after shift)
    in_maps = []
    for c in range(N_CORES):
        pair, role = c // 2, c % 2
        xc = x[pair] if role == 0 else x[pair][::-1]
        in_maps.append({
            "x": np.ascontiguousarray(xc, np.float32),
            "w1": w1b, "w2": w2b, "wrb": wrb, "brb": brb,
            "b1s": b1s, "b2b": b2b, "idp": idp,
        })
    return in_maps


def _assemble(results, x):
    out = np.empty_like(x)
    for pair in range(B):
        a = results[2 * pair]["out"]
        b = results[2 * pair + 1]["out"]
        out[pair, :S // 2] = a[:S // 2]
        out[pair, S // 2:] = b[:S // 2][::-1]
    for c in range(N_CORES):
        dbg = results[c]["dbg"]
        if not (dbg[0, 0] == K_TOP and dbg[0, 1] == K_TOP):
            raise RuntimeError(f"core {c}: top-k count mismatch, dbg={dbg}")
    return out


def run_on_device(x, Wr, br, W1, b1, W2, b2, trace=False, trace_kwargs=None):
    from concourse.bass_utils import run_bass_kernel_spmd
    nc = _build()
    in_maps = _prep_in_maps(x, Wr, br, W1, b1, W2, b2)
    res = run_bass_kernel_spmd(
        nc, in_maps, core_ids=list(range(N_CORES)),
        trace=trace, **(trace_kwargs or {}),
    )
    out = _assemble(res.results, x)
    return out, res


def kernel(x, Wr, br, W1, b1, W2, b2):
    x = np.asarray(x, np.float32)
    out, _ = run_on_device(
        x, np.asarray(Wr, np.float32), np.asarray(br, np.float32),
        np.asarray(W1, np.float32), np.asarray(b1, np.float32),
        np.asarray(W2, np.float32), np.asarray(b2, np.float32))
    return out


# revision 9
# speedup vs baseline: 1.0614x; 1.0099x over previous
"""Trainium2 Bass kernel: Mixture-of-Depths routed FFN block.

Computation (per batch row b of x [B=4, S=4096, D=2048]):
  logits = x[b] @ Wr + br                       # router
  top-512 tokens by logit, positions sorted ascending
  h = gelu(x[b][tokens] @ W1 + b1); o = h @ W2 + b2
  rw = softmax(logits[tokens])
  out[b] = x[b];  out[b][tokens] += rw * o

Distribution (8 NeuronCores, no collectives):
  Core 2p   handles batch p on the natural x[p].
  Core 2p+1 handles batch p on the row-REVERSED x[p] (host flips, then
  un-flips its output half) - this makes the program fully SPMD-symmetric:
  every core routes, selects the top-512, compacts ascending positions in
  its own coordinate space, FFNs the FIRST 384 ranks (covers every token
  that can land in its output half; actual per-half counts are ~256+-25),
  pass-through copies rows [0:2048) of its x to its out, and scatters its
  updated token rows.  Host takes rows [0:2048) from core 2p and reversed
  rows [0:2048) from core 2p+1.

On-device pipeline per core:
  1. Router: stream x tiles [128,2048]; fused multiply+row-reduce on DVE
     against a host-replicated Wr; same tiles pass-through-copied to out.
  2. Top-k threshold: 34-step float bisection on [0.25, 4.0] for the value
     t with count(logit >= t) == 512 (exact; logit sigma == 1 by
     construction so the bracket is safe - verified via the dbg output).
  3. Compaction: slot-space mask -> sparse_gather (GPSIMD) -> sorted token
     ids + their logits; softmax weights from the compacted logits.
  4. Gather 384 token rows via indirect DMA; PE-transpose to d-major bf16.
  5. FFN: hT[dff,tok] = W1.T-tiles @ fxT (bf16, f32 PSUM), fused
     gelu+bias on ACT; out_l[tok,d] = hT.T-tiles @ W2 accumulated in SBUF.
  6. Epilogue: upd = fx + rw*(out_l + b2); indirect-DMA scatter rows.
"""

import numpy as np
import ml_dtypes

B, S, D, DFF = 4, 4096, 2048, 8192
K_TOP = 512
P = 128
WIN = 384            # token ranks processed per core (3 groups of 128)
NG = WIN // P        # 3
N_CORES = 8
BISECT_ITERS = 34
BISECT_LO, BISECT_HI = 0.25, 4.0
MB = 2               # mm1 dff-tile block (PSUM tiles live)
KFB = 4              # mm2 dff-chunk block (W2 tiles live)

_CACHE = {}


def _build():
    if "nc" in _CACHE:
        return _CACHE["nc"]
    from contextlib import ExitStack
    import concourse.bacc as bacc
    import concourse.tile as tile
    from concourse import bass, mybir, library_config
    from concourse.masks import make_identity

    f32 = mybir.dt.float32
    bf16 = mybir.dt.bfloat16
    i32 = mybir.dt.int32
    u32 = mybir.dt.uint32
    A = mybir.AluOpType
    ACTF = mybir.ActivationFunctionType

    nc = bacc.Bacc("TRN2", target_bir_lowering=False, debug=False)

    xd = nc.dram_tensor("x", [S, D], f32, kind="ExternalInput")
    w1d = nc.dram_tensor("w1", [D, DFF], bf16, kind="ExternalInput")
    w2d = nc.dram_tensor("w2", [DFF, D], bf16, kind="ExternalInput")
    wrd = nc.dram_tensor("wrb", [P, D], f32, kind="ExternalInput")
    brd = nc.dram_tensor("brb", [P, 1], f32, kind="ExternalInput")
    b1d = nc.dram_tensor("b1s", [P, DFF // P], f32, kind="ExternalInput")
    b2d = nc.dram_tensor("b2b", [P, D], f32, kind="ExternalInput")
    idd = nc.dram_tensor("idp", [16, S // 16], f32, kind="ExternalInput")
    outd = nc.dram_tensor("out", [S, D], f32, kind="ExternalOutput")
    dbgd = nc.dram_tensor("dbg", [1, 4], f32, kind="ExternalOutput")

    with tile.TileContext(nc) as tc, ExitStack() as ctx:
        const = ctx.enter_context(tc.tile_pool(name="const", bufs=1))
        xpool = ctx.enter_context(tc.tile_pool(name="xp", bufs=3))
        small = ctx.enter_context(tc.tile_pool(name="small", bufs=1))
        bscr = ctx.enter_context(tc.tile_pool(name="bscr", bufs=2))
        fxpool = ctx.enter_context(tc.tile_pool(name="fxp", bufs=1))
        big = ctx.enter_context(tc.tile_pool(name="big", bufs=1))
        w1pool = ctx.enter_context(tc.tile_pool(name="w1p", bufs=8))
        w2pool = ctx.enter_context(tc.tile_pool(name="w2p", bufs=2))
        updp = ctx.enter_context(tc.tile_pool(name="updp", bufs=1))
        pps_ctx = tc.tile_pool(name="pps", bufs=2, space="PSUM")
        pps = pps_ctx.__enter__()

        # ---- constants
        wrb = const.tile([P, D], f32)
        nc.sync.dma_start(wrb[:], wrd[:])
        brb = const.tile([P, 1], f32)
        nc.sync.dma_start(brb[:], brd[:])
        b1s = const.tile([P, DFF // P], f32)
        nc.sync.dma_start(b1s[:], b1d[:])
        b2b = const.tile([P, D], f32)
        nc.sync.dma_start(b2b[:], b2d[:])
        idp = const.tile([16, S // 16], f32)
        nc.sync.dma_start(idp[:], idd[:])
        ident = const.tile([P, P], f32)
        make_identity(nc, ident[:])
        ones_col = const.tile([P, 1], f32)
        nc.vector.memset(ones_col[:], 1.0)
        ones_row = const.tile([1, P], f32)
        nc.vector.memset(ones_row[:], 1.0)
        ones2d = const.tile([P, P], f32)
        nc.vector.memset(ones2d[:], 1.0)

        import contextlib
        scope_stack = []

        def scope(name):
            sid, _ = nc.enter_named_scope(name, False)
            scope_stack.append((name, sid))

        def escope():
            name, sid = scope_stack.pop()
            nc.leave_named_scope(name, sid, False)

        # ---- Phase 1: router
        scope("router")
        L = small.tile([P, S // P], f32)  # L[p, t] = logit(token t*128+p)
        for t in range(S // P):
            xt = xpool.tile([P, D], f32)
            nc.sync.dma_start(xt[:], xd[t * P:(t + 1) * P, :])
            nc.vector.scalar_tensor_tensor(
                out=xt[:], in0=xt[:], scalar=1.0, in1=wrb[:],
                op0=A.mult, op1=A.mult, accum_out=L[:, t:t + 1],
            )
        nc.vector.tensor_scalar(
            out=L[:], in0=L[:], scalar1=brb[:, :1], scalar2=None, op0=A.add)

        escope()
        # ---- HAM keepalive helper: tiny matmul keeps TensorE un-throttled
        def keepalive():
            ka = pps.tile([P, 64], f32, space="PSUM", tag="kp", name="ka")
            nc.tensor.matmul(ka[:], lhsT=ones2d[:], rhs=ones2d[:, :64],
                             start=True, stop=True)

        # ---- Phase 2: bisection for the k-th largest logit
        scope("bisect")
        lo = small.tile([P, 1], f32)
        hi = small.tile([P, 1], f32)
        mid = small.tile([P, 1], f32)
        ge = small.tile([P, 1], mybir.dt.uint8)
        lt = small.tile([P, 1], mybir.dt.uint8)
        nc.vector.memset(lo[:], BISECT_LO)
        nc.vector.memset(hi[:], BISECT_HI)
        for _ in range(BISECT_ITERS):
            nc.vector.tensor_tensor(out=mid[:], in0=lo[:], in1=hi[:], op=A.add)
            nc.vector.tensor_scalar_mul(mid[:], mid[:], 0.5)
            scrb = bscr.tile([P, S // P], f32)
            cnt = bscr.tile([P, 1], f32)
            nc.vector.tensor_scalar(
                out=scrb[:], in0=L[:], scalar1=mid[:, :1], scalar2=None,
                op0=A.is_ge, op1=A.add, accum_out=cnt[:, :1],
            )
            bc = pps.tile([P, 1], f32, space="PSUM", tag="pP1")
            nc.tensor.matmul(bc[:], lhsT=ones2d[:], rhs=cnt[:], start=True, stop=True)
            nc.vector.tensor_scalar(
                out=ge[:], in0=bc[:], scalar1=float(K_TOP), scalar2=None, op0=A.is_ge)
            nc.vector.copy_predicated(lo[:], ge[:], mid[:])
            nc.vector.tensor_scalar(
                out=lt[:], in0=bc[:], scalar1=float(K_TOP), scalar2=None, op0=A.is_lt)
            nc.vector.copy_predicated(hi[:], lt[:], mid[:])
            keepalive()

        escope()
        # ---- Phase 3: compaction (slot space: token i at [i%16, i//16])
        scope("compact")
        L16 = small.tile([16, S // 16], f32)
        for q in range(8):
            nc.sync.dma_start(L16[:, q:S // 16:8], L[16 * q:16 * (q + 1), :])
        t16 = lo[0:16, 0:1]
        arr_ids = small.tile([16, S // 16], f32)
        nc.vector.scalar_tensor_tensor(
            out=arr_ids[:], in0=L16[:], scalar=t16, in1=idp[:],
            op0=A.is_ge, op1=A.mult)
        nc.vector.tensor_scalar_add(arr_ids[:], arr_ids[:], -1.0)
        lp64 = small.tile([16, S // 16], f32)
        nc.vector.tensor_scalar_add(lp64[:], L16[:], 64.0)
        arr_lg = small.tile([16, S // 16], f32)
        nc.vector.scalar_tensor_tensor(
            out=arr_lg[:], in0=L16[:], scalar=t16, in1=lp64[:],
            op0=A.is_ge, op1=A.mult)
        nc.vector.tensor_scalar_add(arr_lg[:], arr_lg[:], -1.0)
        ids512 = small.tile([16, K_TOP // 16], f32)
        lg512 = small.tile([16, K_TOP // 16], f32)
        nf1 = small.tile([1, 1], u32)
        nf2 = small.tile([1, 1], u32)
        with tc.tile_critical():
            nc.gpsimd.load_library(library_config.sparse_gather)
            nc.gpsimd.sparse_gather(ids512[:], arr_ids[:], num_found=nf1[:, :1])
            nc.gpsimd.sparse_gather(lg512[:], arr_lg[:], num_found=nf2[:, :1])
        for _ in range(4):
            keepalive()
        dbg = small.tile([1, 4], f32)
        nc.vector.tensor_copy(dbg[:, 0:1], nf1[:])
        nc.vector.tensor_copy(dbg[:, 1:2], nf2[:])
        nc.vector.tensor_copy(dbg[:, 2:3], lo[0:1, 0:1])
        nc.vector.tensor_copy(dbg[:, 3:4], hi[0:1, 0:1])
        nc.sync.dma_start(dbgd[:], dbg[:])

        escope()
        # ---- Phase 4: token-major shuffles, softmax weights, int ids
        scope("softmax")
        ids_tok = small.tile([P, K_TOP // P], f32)
        lg_tok = small.tile([P, K_TOP // P], f32)
        for q in range(8):
            nc.sync.dma_start(ids_tok[16 * q:16 * (q + 1), :], ids512[:, q:K_TOP // 16:8])
            nc.sync.dma_start(lg_tok[16 * q:16 * (q + 1), :], lg512[:, q:K_TOP // 16:8])
        nc.vector.tensor_scalar_add(lg_tok[:], lg_tok[:], -63.0)
        exps = small.tile([P, K_TOP // P], f32)
        nc.scalar.activation(exps[:], lg_tok[:], ACTF.Exp)
        sume = small.tile([P, 1], f32)
        nc.vector.tensor_reduce(sume[:], exps[:], axis=mybir.AxisListType.X, op=A.add)
        den = pps.tile([1, 1], f32, space="PSUM", tag="p11")
        nc.tensor.matmul(den[:], lhsT=ones_col[:], rhs=sume[:], start=True, stop=True)
        den_sb = small.tile([1, 1], f32)
        nc.vector.tensor_copy(den_sb[:], den[:])
        recip = small.tile([1, 1], f32)
        nc.vector.reciprocal(recip[:], den_sb[:])
        rcb = pps.tile([P, 1], f32, space="PSUM", tag="pP1")
        nc.tensor.matmul(rcb[:], lhsT=ones_row[:], rhs=recip[:], start=True, stop=True)
        rw_tok = small.tile([P, NG], f32)
        nc.vector.tensor_scalar_mul(rw_tok[:], exps[:, :NG], rcb[:, :1])
        for _ in range(4):
            keepalive()
        ids_i32 = small.tile([P, NG], i32)
        nc.vector.tensor_copy(ids_i32[:], ids_tok[:, :NG])

        escope()
        # ---- Phase 5: gather + transpose to fxT (bf16, d-major)
        scope("gathertr")
        pps_ctx.__exit__(None, None, None)
        mmp = ctx.enter_context(tc.tile_pool(name="mmp", bufs=3, space="PSUM"))
        mm2p = ctx.enter_context(tc.tile_pool(name="mm2p", bufs=2, space="PSUM"))
        fxT = [big.tile([P, WIN], bf16, name=f"fxT{k}") for k in range(D // P)]
        fx_list = []
        for g in range(NG):
            fxg = fxpool.tile([P, D], f32, tag=f"fx{g}", name=f"fxg{g}")
            nc.gpsimd.indirect_dma_start(
                out=fxg[:], out_offset=None, in_=xd[:],
                in_offset=bass.IndirectOffsetOnAxis(ap=ids_i32[:, g:g + 1], axis=0),
            )
            fx_list.append(fxg)
            for k in range(16):
                tp = mm2p.tile([P, P], f32, space="PSUM", tag="ps", name="tp")
                nc.tensor.transpose(tp[:], fxg[:, k * P:(k + 1) * P], ident[:])
                nc.vector.tensor_copy(fxT[k][:, g * P:(g + 1) * P], tp[:])

        escope()
        # ---- pass-through copy (DRAM->DRAM), overlaps the FFN phase
        scope("passthru")
        for t in range(S // P // 2):
            nc.sync.dma_start(outd[t * P:(t + 1) * P, :], xd[t * P:(t + 1) * P, :])

        escope()
        # ---- Phase 6: mm1 + fused gelu/bias -> hT (bf16, dff-major)
        scope("mm1")
        hT = [big.tile([P, WIN], bf16, name=f"hT{m}") for m in range(DFF // P)]
        for mb in range(DFF // P // MB):
            hps = [mmp.tile([P, WIN], f32, space="PSUM", tag=f"hps{j}", name=f"hps{j}") for j in range(MB)]
            for k in range(D // P):
                w1sl = w1pool.tile([P, MB * P], bf16)
                nc.sync.dma_start(
                    w1sl[:], w1d[k * P:(k + 1) * P, mb * MB * P:(mb + 1) * MB * P])
                for j in range(MB):
                    nc.tensor.matmul(
                        hps[j][:], lhsT=w1sl[:, j * P:(j + 1) * P],
                        rhs=fxT[k][:],
                        start=(k == 0), stop=(k == D // P - 1))
            for j in range(MB):
                m = mb * MB + j
                nc.scalar.activation(
                    hT[m][:], hps[j][:], ACTF.Gelu, bias=b1s[:, m:m + 1])

        escope()
        # ---- Phase 7: mm2, accumulate out_l in SBUF
        scope("mm2")
        acc = big.tile([P, NG * D], f32)  # token group g -> [:, g*D:(g+1)*D]
        n_kfb = DFF // P // KFB
        for kfb in range(n_kfb):
            w2t = [w2pool.tile([P, D], bf16, tag=f"w2{j}", name=f"w2t{j}") for j in range(KFB)]
            for j in range(KFB):
                kf = kfb * KFB + j
                nc.sync.dma_start(w2t[j][:], w2d[kf * P:(kf + 1) * P, :])
            for g in range(NG):
                for ds in range(D // 512):
                    ps = mm2p.tile([P, 512], f32, space="PSUM", tag="ps")
                    for j in range(KFB):
                        kf = kfb * KFB + j
                        nc.tensor.matmul(
                            ps[:], lhsT=hT[kf][:, g * P:(g + 1) * P],
                            rhs=w2t[j][:, ds * 512:(ds + 1) * 512],
                            start=(j == 0), stop=(j == KFB - 1))
                    dst = acc[:, g * D + ds * 512:g * D + (ds + 1) * 512]
                    if kfb == 0:
                        nc.vector.tensor_copy(dst, ps[:])
                    else:
                        nc.vector.tensor_tensor(out=dst, in0=dst, in1=ps[:], op=A.add)

        escope()
        # ---- Phase 8: epilogue + scatter
        scope("epilogue")
        for g in range(NG):
            upd = updp.tile([P, D], f32)
            nc.vector.tensor_tensor(
                out=upd[:], in0=acc[:, g * D:(g + 1) * D], in1=b2b[:], op=A.add)
            nc.vector.scalar_tensor_tensor(
                out=upd[:], in0=upd[:], scalar=rw_tok[:, g:g + 1], in1=fx_list[g][:],
                op0=A.mult, op1=A.add)
            nc.gpsimd.indirect_dma_start(
                out=outd[:],
                out_offset=bass.IndirectOffsetOnAxis(ap=ids_i32[:, g:g + 1], axis=0),
                in_=upd[:], in_offset=None,
            )

    if scope_stack:
        escope()
    nc.compile()
    _CACHE["nc"] = nc
    return nc


def _prep_in_maps(x, Wr, br, W1, b1, W2, b2):
    bf = ml_dtypes.bfloat16
    w1b = np.ascontiguousarray(W1.astype(bf))
    w2b = np.ascontiguousarray(W2.astype(bf))
    wrb = np.ascontiguousarray(np.broadcast_to(Wr[:, 0][None, :], (P, D)), np.float32)
    brb = np.full((P, 1), np.float32(br[0]), np.float32)
    b1s = np.ascontiguousarray(b1.reshape(DFF // P, P).T, np.float32)
    b2b = np.ascontiguousarray(np.broadcast_to(b2[None, :], (P, D)), np.float32)
    sl = np.arange(S)
    idp = np.zeros((16, S // 16), np.float32)
    idp[sl % 16, sl // 16] = sl + 1  # slot id + 1 (so unselected -> -1 after shift)
    in_maps = []
    for c in range(N_CORES):
        pair, role = c // 2, c % 2
        xc = x[pair] if role == 0 else x[pair][::-1]
        in_maps.append({
            "x": np.ascontiguousarray(xc, np.float32),
            "w1": w1b, "w2": w2b, "wrb": wrb, "brb": brb,
            "b1s": b1s, "b2b": b2b, "idp": idp,
        })
    return in_maps


def _assemble(results, x):
    out = np.empty_like(x)
    for pair in range(B):
        a = results[2 * pair]["out"]
        b = results[2 * pair + 1]["out"]
        out[pair, :S // 2] = a[:S // 2]
        out[pair, S // 2:] = b[:S // 2][::-1]
    for c in range(N_CORES):
        dbg = results[c]["dbg"]
        if not (dbg[0, 0] == K_TOP and dbg[0, 1] == K_TOP):
            raise RuntimeError(f"core {c}: top-k count mismatch, dbg={dbg}")
    return out


def run_on_device(x, Wr, br, W1, b1, W2, b2, trace=False, trace_kwargs=None):
    from concourse.bass_utils import run_bass_kernel_spmd
    nc = _build()
    in_maps = _prep_in_maps(x, Wr, br, W1, b1, W2, b2)
    res = run_bass_kernel_spmd(
        nc, in_maps, core_ids=list(range(N_CORES)),
        trace=trace, **(trace_kwargs or {}),
    )
    out = _assemble(res.results, x)
    return out, res


def kernel(x, Wr, br, W1, b1, W2, b2):
    x = np.asarray(x, np.float32)
    out, _ = run_on_device(
        x, np.asarray(Wr, np.float32), np.asarray(br, np.float32),
        np.asarray(W1, np.float32), np.asarray(b1, np.float32),
        np.asarray(W2, np.float32), np.asarray(b2, np.float32))
    return out


# revision 13
# speedup vs baseline: 1.0808x; 1.0183x over previous
"""Trainium2 Bass kernel: Mixture-of-Depths routed FFN block.

Computation (per batch row b of x [B=4, S=4096, D=2048]):
  logits = x[b] @ Wr + br                       # router
  top-512 tokens by logit, positions sorted ascending
  h = gelu(x[b][tokens] @ W1 + b1); o = h @ W2 + b2
  rw = softmax(logits[tokens])
  out[b] = x[b];  out[b][tokens] += rw * o

Distribution (8 NeuronCores, no collectives):
  Core 2p   handles batch p on the natural x[p].
  Core 2p+1 handles batch p on the row-REVERSED x[p] (host flips, then
  un-flips its output half) - this makes the program fully SPMD-symmetric:
  every core routes, selects the top-512, compacts ascending positions in
  its own coordinate space, FFNs the FIRST 384 ranks (covers every token
  that can land in its output half; actual per-half counts are ~256+-25),
  pass-through copies rows [0:2048) of its x to its out, and scatters its
  updated token rows.  Host takes rows [0:2048) from core 2p and reversed
  rows [0:2048) from core 2p+1.

On-device pipeline per core:
  1. Router: stream x tiles [128,2048]; fused multiply+row-reduce on DVE
     against a host-replicated Wr; same tiles pass-through-copied to out.
  2. Top-k threshold: 34-step float bisection on [0.25, 4.0] for the value
     t with count(logit >= t) == 512 (exact; logit sigma == 1 by
     construction so the bracket is safe - verified via the dbg output).
  3. Compaction: slot-space mask -> sparse_gather (GPSIMD) -> sorted token
     ids + their logits; softmax weights from the compacted logits.
  4. Gather 384 token rows via indirect DMA; PE-transpose to d-major bf16.
  5. FFN: hT[dff,tok] = W1.T-tiles @ fxT (bf16, f32 PSUM), fused
     gelu+bias on ACT; out_l[tok,d] = hT.T-tiles @ W2 accumulated in SBUF.
  6. Epilogue: upd = fx + rw*(out_l + b2); indirect-DMA scatter rows.
"""

import numpy as np
import ml_dtypes

B, S, D, DFF = 4, 4096, 2048, 8192
K_TOP = 512
P = 128
WIN = 384            # token ranks processed per core (3 groups of 128)
NG = WIN // P        # 3
N_CORES = 8
BISECT_ITERS = 34
BISECT_LO, BISECT_HI = 0.25, 4.0
MB = 2               # mm1 dff-tile block (PSUM tiles live)
KFB = 4              # mm2 dff-chunk block (W2 tiles live)

_CACHE = {}


def _build():
    if "nc" in _CACHE:
        return _CACHE["nc"]
    from contextlib import ExitStack
    import concourse.bacc as bacc
    import concourse.tile as tile
    from concourse import bass, mybir, library_config
    from concourse.masks import make_identity

    f32 = mybir.dt.float32
    bf16 = mybir.dt.bfloat16
    i32 = mybir.dt.int32
    u32 = mybir.dt.uint32
    A = mybir.AluOpType
    ACTF = mybir.ActivationFunctionType

    nc = bacc.Bacc("TRN2", target_bir_lowering=False, debug=False)

    xd = nc.dram_tensor("x", [S, D], f32, kind="ExternalInput")
    w1d = nc.dram_tensor("w1", [D, DFF], bf16, kind="ExternalInput")
    w2d = nc.dram_tensor("w2", [DFF, D], bf16, kind="ExternalInput")
    wrd = nc.dram_tensor("wrb", [P, D], f32, kind="ExternalInput")
    brd = nc.dram_tensor("brb", [P, 1], f32, kind="ExternalInput")
    b1d = nc.dram_tensor("b1s", [P, DFF // P], f32, kind="ExternalInput")
    b2d = nc.dram_tensor("b2b", [P, D], f32, kind="ExternalInput")
    idd = nc.dram_tensor("idp", [16, S // 16], f32, kind="ExternalInput")
    outd = nc.dram_tensor("out", [S, D], f32, kind="ExternalOutput")
    dbgd = nc.dram_tensor("dbg", [1, 4], f32, kind="ExternalOutput")

    with tile.TileContext(nc) as tc, ExitStack() as ctx:
        const = ctx.enter_context(tc.tile_pool(name="const", bufs=1))
        xpool = ctx.enter_context(tc.tile_pool(name="xp", bufs=3))
        small = ctx.enter_context(tc.tile_pool(name="small", bufs=1))
        bscr = ctx.enter_context(tc.tile_pool(name="bscr", bufs=2))
        fxpool = ctx.enter_context(tc.tile_pool(name="fxp", bufs=1))
        big = ctx.enter_context(tc.tile_pool(name="big", bufs=1))
        w1pool = ctx.enter_context(tc.tile_pool(name="w1p", bufs=8))
        w2pool = ctx.enter_context(tc.tile_pool(name="w2p", bufs=2))
        pps_ctx = tc.tile_pool(name="pps", bufs=2, space="PSUM")
        pps = pps_ctx.__enter__()

        # ---- constants
        wrb = const.tile([P, D], f32)
        nc.sync.dma_start(wrb[:], wrd[:])
        brb = const.tile([P, 1], f32)
        nc.sync.dma_start(brb[:], brd[:])
        b1s = const.tile([P, DFF // P], f32)
        nc.sync.dma_start(b1s[:], b1d[:])
        b2b = const.tile([P, D], f32)
        nc.sync.dma_start(b2b[:], b2d[:])
        idp = const.tile([16, S // 16], f32)
        nc.sync.dma_start(idp[:], idd[:])
        ident = const.tile([P, P], f32)
        make_identity(nc, ident[:])
        ones_col = const.tile([P, 1], f32)
        nc.vector.memset(ones_col[:], 1.0)
        ones_row = const.tile([1, P], f32)
        nc.vector.memset(ones_row[:], 1.0)
        ones2d = const.tile([P, P], f32)
        nc.vector.memset(ones2d[:], 1.0)

        import contextlib
        scope_stack = []

        def scope(name):
            sid, _ = nc.enter_named_scope(name, False)
            scope_stack.append((name, sid))

        def escope():
            name, sid = scope_stack.pop()
            nc.leave_named_scope(name, sid, False)

        # ---- Phase 1: router
        scope("router")
        L = small.tile([P, S // P], f32)  # L[p, t] = logit(token t*128+p)
        for t in range(S // P):
            xt = xpool.tile([P, D], f32)
            nc.sync.dma_start(xt[:], xd[t * P:(t + 1) * P, :])
            nc.vector.scalar_tensor_tensor(
                out=xt[:], in0=xt[:], scalar=1.0, in1=wrb[:],
                op0=A.mult, op1=A.mult, accum_out=L[:, t:t + 1],
            )
        nc.vector.tensor_scalar(
            out=L[:], in0=L[:], scalar1=brb[:, :1], scalar2=None, op0=A.add)

        escope()
        # ---- HAM keepalive helper: tiny matmul keeps TensorE un-throttled
        def keepalive():
            ka = pps.tile([P, 64], f32, space="PSUM", tag="kp", name="ka")
            nc.tensor.matmul(ka[:], lhsT=ones2d[:], rhs=ones2d[:, :64],
                             start=True, stop=True)

        # ---- Phase 2: bisection for the k-th largest logit
        scope("bisect")
        lo = small.tile([P, 1], f32)
        hi = small.tile([P, 1], f32)
        mid = small.tile([P, 1], f32)
        ge = small.tile([P, 1], mybir.dt.uint8)
        lt = small.tile([P, 1], mybir.dt.uint8)
        nc.vector.memset(lo[:], BISECT_LO)
        nc.vector.memset(hi[:], BISECT_HI)
        for _ in range(BISECT_ITERS):
            nc.vector.tensor_tensor(out=mid[:], in0=lo[:], in1=hi[:], op=A.add)
            nc.vector.tensor_scalar_mul(mid[:], mid[:], 0.5)
            scrb = bscr.tile([P, S // P], f32)
            cnt = bscr.tile([P, 1], f32)
            nc.vector.tensor_scalar(
                out=scrb[:], in0=L[:], scalar1=mid[:, :1], scalar2=None,
                op0=A.is_ge, op1=A.add, accum_out=cnt[:, :1],
            )
            bc = pps.tile([P, 1], f32, space="PSUM", tag="pP1")
            nc.tensor.matmul(bc[:], lhsT=ones2d[:], rhs=cnt[:], start=True, stop=True)
            nc.vector.tensor_scalar(
                out=ge[:], in0=bc[:], scalar1=float(K_TOP), scalar2=None, op0=A.is_ge)
            nc.vector.copy_predicated(lo[:], ge[:], mid[:])
            nc.vector.tensor_scalar(
                out=lt[:], in0=bc[:], scalar1=float(K_TOP), scalar2=None, op0=A.is_lt)
            nc.vector.copy_predicated(hi[:], lt[:], mid[:])
            keepalive()

        escope()
        # ---- Phase 3: compaction (slot space: token i at [i%16, i//16])
        scope("compact")
        L16 = small.tile([16, S // 16], f32)
        for q in range(8):
            nc.sync.dma_start(L16[:, q:S // 16:8], L[16 * q:16 * (q + 1), :])
        t16 = lo[0:16, 0:1]
        arr_ids = small.tile([16, S // 16], f32)
        nc.vector.scalar_tensor_tensor(
            out=arr_ids[:], in0=L16[:], scalar=t16, in1=idp[:],
            op0=A.is_ge, op1=A.mult)
        nc.vector.tensor_scalar_add(arr_ids[:], arr_ids[:], -1.0)
        lp64 = small.tile([16, S // 16], f32)
        nc.vector.tensor_scalar_add(lp64[:], L16[:], 64.0)
        arr_lg = small.tile([16, S // 16], f32)
        nc.vector.scalar_tensor_tensor(
            out=arr_lg[:], in0=L16[:], scalar=t16, in1=lp64[:],
            op0=A.is_ge, op1=A.mult)
        nc.vector.tensor_scalar_add(arr_lg[:], arr_lg[:], -1.0)
        ids512 = small.tile([16, K_TOP // 16], f32)
        lg512 = small.tile([16, K_TOP // 16], f32)
        nf1 = small.tile([1, 1], u32)
        nf2 = small.tile([1, 1], u32)
        with tc.tile_critical():
            nc.gpsimd.load_library(library_config.sparse_gather)
            nc.gpsimd.sparse_gather(ids512[:], arr_ids[:], num_found=nf1[:, :1])
            nc.gpsimd.sparse_gather(lg512[:], arr_lg[:], num_found=nf2[:, :1])
        for _ in range(4):
            keepalive()
        dbg = small.tile([1, 4], f32)
        nc.vector.tensor_copy(dbg[:, 0:1], nf1[:])
        nc.vector.tensor_copy(dbg[:, 1:2], nf2[:])
        nc.vector.tensor_copy(dbg[:, 2:3], lo[0:1, 0:1])
        nc.vector.tensor_copy(dbg[:, 3:4], hi[0:1, 0:1])
        nc.sync.dma_start(dbgd[:], dbg[:])

        escope()
        # ---- Phase 4: token-major shuffles, softmax weights, int ids
        scope("softmax")
        lg_tok = small.tile([P, K_TOP // P], f32)
        for q in range(8):
            nc.sync.dma_start(lg_tok[16 * q:16 * (q + 1), :], lg512[:, q:K_TOP // 16:8])
        ids16 = small.tile([16, K_TOP // 16], mybir.dt.int16)
        nc.vector.tensor_copy(ids16[:], ids512[:])
        ids128 = small.tile([P, WIN // 16], mybir.dt.int16)
        for q in range(8):
            nc.sync.dma_start(ids128[16 * q:16 * (q + 1), :], ids16[:, :WIN // 16])
        nc.vector.tensor_scalar_add(lg_tok[:], lg_tok[:], -63.0)
        exps = small.tile([P, K_TOP // P], f32)
        nc.scalar.activation(exps[:], lg_tok[:], ACTF.Exp)
        sume = small.tile([P, 1], f32)
        nc.vector.tensor_reduce(sume[:], exps[:], axis=mybir.AxisListType.X, op=A.add)
        den = pps.tile([1, 1], f32, space="PSUM", tag="p11")
        nc.tensor.matmul(den[:], lhsT=ones_col[:], rhs=sume[:], start=True, stop=True)
        den_sb = small.tile([1, 1], f32)
        nc.vector.tensor_copy(den_sb[:], den[:])
        recip = small.tile([1, 1], f32)
        nc.vector.reciprocal(recip[:], den_sb[:])
        rcb = pps.tile([P, 1], f32, space="PSUM", tag="pP1")
        nc.tensor.matmul(rcb[:], lhsT=ones_row[:], rhs=recip[:], start=True, stop=True)
        rw_tok = small.tile([P, NG], f32)
        nc.vector.tensor_scalar_mul(rw_tok[:], exps[:, :NG], rcb[:, :1])
        for _ in range(4):
            keepalive()

        escope()
        # ---- Phase 5: gather + transpose to fxT (bf16, d-major)
        scope("gathertr")
        pps_ctx.__exit__(None, None, None)
        mmp = ctx.enter_context(tc.tile_pool(name="mmp", bufs=3, space="PSUM"))
        mm2p = ctx.enter_context(tc.tile_pool(name="mm2p", bufs=2, space="PSUM"))
        fxT = [big.tile([P, WIN], bf16, name=f"fxT{k}") for k in range(D // P)]
        fx3 = fxpool.tile([P, NG, D], f32, name="fx3")
        gsem = nc.alloc_semaphore("fx_gather_dma")
        gprep = nc.alloc_semaphore("fx_gather_prep")
        with tc.tile_critical():
            nc.gpsimd.load_library(library_config.mlp)
            nc.gpsimd.dma_gather(
                fx3[:], xd[:], ids128[:], WIN, WIN, D,
                prepare_only=True, sem=gsem,
            ).then_inc(gprep, 1)
            nc.gpsimd.wait_ge(gprep, 1)
            nc.gpsimd.trigger_dma(count=1)
            nc.gpsimd.wait_ge(gsem, 16)
        for g in range(NG):
            for k in range(16):
                tp = mm2p.tile([P, P], f32, space="PSUM", tag="ps", name="tp")
                nc.tensor.transpose(tp[:], fx3[:, g, k * P:(k + 1) * P], ident[:])
                nc.vector.tensor_copy(fxT[k][:, g * P:(g + 1) * P], tp[:])

        escope()
        # ---- pass-through copy (DRAM->DRAM), overlaps the FFN phase
        scope("passthru")
        for t in range(S // P // 2):
            nc.sync.dma_start(outd[t * P:(t + 1) * P, :], xd[t * P:(t + 1) * P, :])

        escope()
        # ---- Phase 6: mm1 + fused gelu/bias -> hT (bf16, dff-major)
        scope("mm1")
        hT = [big.tile([P, WIN], bf16, name=f"hT{m}") for m in range(DFF // P)]
        for mb in range(DFF // P // MB):
            hps = [mmp.tile([P, WIN], f32, space="PSUM", tag=f"hps{j}", name=f"hps{j}") for j in range(MB)]
            for k in range(D // P):
                w1sl = w1pool.tile([P, MB * P], bf16)
                nc.sync.dma_start(
                    w1sl[:], w1d[k * P:(k + 1) * P, mb * MB * P:(mb + 1) * MB * P])
                for j in range(MB):
                    nc.tensor.matmul(
                        hps[j][:], lhsT=w1sl[:, j * P:(j + 1) * P],
                        rhs=fxT[k][:],
                        start=(k == 0), stop=(k == D // P - 1))
            for j in range(MB):
                m = mb * MB + j
                nc.scalar.activation(
                    hT[m][:], hps[j][:], ACTF.Gelu, bias=b1s[:, m:m + 1])

        escope()
        # ---- Phase 7: mm2, accumulate out_l in SBUF
        scope("mm2")
        acc = big.tile([P, NG * D], f32)  # token group g -> [:, g*D:(g+1)*D]
        n_kfb = DFF // P // KFB
        for kfb in range(n_kfb):
            w2t = [w2pool.tile([P, D], bf16, tag=f"w2{j}", name=f"w2t{j}") for j in range(KFB)]
            for j in range(KFB):
                kf = kfb * KFB + j
                nc.sync.dma_start(w2t[j][:], w2d[kf * P:(kf + 1) * P, :])
            for g in range(NG):
                for ds in range(D // 512):
                    ps = mm2p.tile([P, 512], f32, space="PSUM", tag="ps")
                    for j in range(KFB):
                        kf = kfb * KFB + j
                        nc.tensor.matmul(
                            ps[:], lhsT=hT[kf][:, g * P:(g + 1) * P],
                            rhs=w2t[j][:, ds * 512:(ds + 1) * 512],
                            start=(j == 0), stop=(j == KFB - 1))
                    dst = acc[:, g * D + ds * 512:g * D + (ds + 1) * 512]
                    if kfb == 0:
                        nc.vector.tensor_copy(dst, ps[:])
                    else:
                        nc.vector.tensor_tensor(out=dst, in0=dst, in1=ps[:], op=A.add)

        escope()
        # ---- Phase 8: epilogue + scatter
        scope("epilogue")
        for g in range(NG):
            sl = acc[:, g * D:(g + 1) * D]
            nc.vector.tensor_tensor(out=sl, in0=sl, in1=b2b[:], op=A.add)
            nc.vector.tensor_scalar_mul(sl, sl, rw_tok[:, g:g + 1])
        ssem = nc.alloc_semaphore("scatter_dma")
        sprep = nc.alloc_semaphore("scatter_prep")
        with tc.tile_critical():
            nc.gpsimd.dma_scatter_add(
                outd[:], acc[:].rearrange("p (g d) -> p g d", d=D),
                ids128[:], WIN, WIN, D,
                prepare_only=True, sem=ssem,
            ).then_inc(sprep, 1)
            nc.gpsimd.wait_ge(sprep, 1)
            nc.gpsimd.trigger_dma(count=1)
            nc.gpsimd.wait_ge(ssem, 16)

    if scope_stack:
        escope()
    nc.compile()
    _CACHE["nc"] = nc
    return nc


def _prep_in_maps(x, Wr, br, W1, b1, W2, b2):
    bf = ml_dtypes.bfloat16
    w1b = np.ascontiguousarray(W1.astype(bf))
    w2b = np.ascontiguousarray(W2.astype(bf))
    wrb = np.ascontiguousarray(np.broadcast_to(Wr[:, 0][None, :], (P, D)), np.float32)
    brb = np.full((P, 1), np.float32(br[0]), np.float32)
    b1s = np.ascontiguousarray(b1.reshape(DFF // P, P).T, np.float32)
    b2b = np.ascontiguousarray(np.broadcast_to(b2[None, :], (P, D)), np.float32)
    sl = np.arange(S)
    idp = np.zeros((16, S // 16), np.float32)
    idp[sl % 16, sl // 16] = sl + 1  # slot id + 1 (so unselected -> -1 after shift)
    in_maps = []
    for c in range(N_CORES):
        pair, role = c // 2, c % 2
        xc = x[pair] if role == 0 else x[pair][::-1]
        in_maps.append({
            "x": np.ascontiguousarray(xc, np.float32),
            "w1": w1b, "w2": w2b, "wrb": wrb, "brb": brb,
            "b1s": b1s, "b2b": b2b, "idp": idp,
        })
    return in_maps


def _assemble(results, x):
    out = np.empty_like(x)
    for pair in range(B):
        a = results[2 * pair]["out"]
        b = results[2 * pair + 1]["out"]
        out[pair, :S // 2] = a[:S // 2]
        out[pair, S // 2:] = b[:S // 2][::-1]
    for c in range(N_CORES):
        dbg = results[c]["dbg"]
        if not (dbg[0, 0] == K_TOP and dbg[0, 1] == K_TOP):
            raise RuntimeError(f"core {c}: top-k count mismatch, dbg={dbg}")
    return out


def run_on_device(x, Wr, br, W1, b1, W2, b2, trace=False, trace_kwargs=None):
    from concourse.bass_utils import run_bass_kernel_spmd
    nc = _build()
    in_maps = _prep_in_maps(x, Wr, br, W1, b1, W2, b2)
    res = run_bass_kernel_spmd(
        nc, in_maps, core_ids=list(range(N_CORES)),
        trace=trace, **(trace_kwargs or {}),
    )
    out = _assemble(res.results, x)
    return out, res


def kernel(x, Wr, br, W1, b1, W2, b2):
    x = np.asarray(x, np.float32)
    out, _ = run_on_device(
        x, np.asarray(Wr, np.float32), np.asarray(br, np.float32),
        np.asarray(W1, np.float32), np.asarray(b1, np.float32),
        np.asarray(W2, np.float32), np.asarray(b2, np.float32))
    return out
